# revision 5
# baseline (speedup 1.0000x reference)
"""Trainium2 Bass kernel for single-head causal attention.

Problem: B=4, S=4096, d_in=512, d_out=64 (fp32 reference).

Sharding (8 cores): core c = (batch b = c//2, query-parity h = c%2).
Each core handles one batch and the 16 query blocks of 128 with block
index === h (mod 2).  The host permutes the sequence dimension so each
core's x^T arrives as [own 2048 query columns | other 2048], which makes
the SPMD program identical across cores: all parity differences are
encoded in a per-core multiplicative mask input.

Device program per core:
  - project [Wk|Wq] against x^T chunks -> qk_sb [128, 4096] fp16
    (rows 0:64 = k^T, rows 64:128 = q^T, columns in local order)
  - project Wv -> v_sb [128, 32*65] fp16 (V blocks of [128, 64] plus a
    fused ones column per block for the softmax denominator)
  - attention in two sweeps of 4 query groups (group = 256 queries):
    for each key block kb: scores^T = k_kb^T.T @ q_group (PSUM, fp32),
    one Exp activation over all valid groups (scale = 1/8, no max
    subtraction -- scores are bounded), multiplicative causal masks on
    the <=1 masked group, then P-stationary matmuls accumulate
    out[q,0:64] = P.T @ V and out[q,64] = sum_k P into per-group PSUM
    slots.  Finalize = reciprocal + per-partition scale + DMA out.
"""

import os
import sys

sys.path.insert(0, "/opt/trn_rl_repo")

import numpy as np

import concourse.bass as bass
import concourse.mybir as mybir
import concourse.tile as tile

B, S, DIN, DOUT = 4, 4096, 512, 64
NCORES = 8
NBLK = S // 128          # 32 key blocks of 128
NGRP = 8                 # query groups of 256 (own queries only)
F16 = mybir.dt.float16
F32 = mybir.dt.float32


def split_waits(nc, maxw=1):
    """Walrus in this toolchain rejects >1 semaphore wait on ctrl-class
    instructions; hoist excess waits onto preceding same-engine NoOps."""
    ctr = 0
    for f in nc.m.functions:
        for bb in f.blocks:
            out = []
            for inst in bb.instructions:
                si = inst.sync_info
                waits = list(si.on_wait) if si and si.on_wait else []
                if len(waits) > maxw:
                    hoisted, rest = waits[:-maxw], waits[-maxw:]
                    for i in range(0, len(hoisted), maxw):
                        nop = mybir.InstNoOp(
                            name=f"waitsplit-{ctr}",
                            ins=[],
                            outs=[],
                            engine=inst.engine,
                            sync_info=mybir.SyncInfo(
                                on_wait=hoisted[i : i + maxw], on_update=[]
                            ),
                        )
                        ctr += 1
                        out.append(nop)
                    si.on_wait = rest
                out.append(inst)
            bb.instructions = out
    return ctr


def build_program(reps=1, patch=True):
    nc = bass.Bass("TRN2", target_bir_lowering=False, debug=False)

    xt = nc.dram_tensor("xt", [DIN, S], F16, kind="ExternalInput")
    wkq = nc.dram_tensor("wkq", [DIN, 128], F16, kind="ExternalInput")
    wv = nc.dram_tensor("wv", [DIN, DOUT], F16, kind="ExternalInput")
    masks = nc.dram_tensor("masks", [4, 128, 256], F16, kind="ExternalInput")
    out = nc.dram_tensor("out", [S // 2, DOUT], F32, kind="ExternalOutput")

    with tile.TileContext(nc) as tc:
        with (
            tc.tile_pool(name="const", bufs=1) as cpool,
            tc.tile_pool(name="big", bufs=1) as bigpool,
            tc.tile_pool(name="xt", bufs=2) as xtpool,
            tc.tile_pool(name="p", bufs=3) as ppool,
            tc.tile_pool(name="osb", bufs=3) as opool,
            tc.tile_pool(name="sc", bufs=3, space="PSUM") as scpool,
            tc.tile_pool(name="acc", bufs=2, space="PSUM") as accpool,
        ):
            def body(_iv=None):
                w_kq = cpool.tile([128, 512], F16, tag="wkq")
                nc.sync.dma_start(
                    out=w_kq[:].rearrange("p (i m) -> p i m", i=4),
                    in_=wkq[:].rearrange("(i p) m -> p i m", p=128),
                )
                w_v = cpool.tile([128, 256], F16, tag="wv")
                nc.sync.dma_start(
                    out=w_v[:].rearrange("p (i m) -> p i m", i=4),
                    in_=wv[:].rearrange("(i p) m -> p i m", p=128),
                )
                mk = cpool.tile([128, 1024], F16, tag="mk")
                nc.sync.dma_start(
                    out=mk[:].rearrange("p (r m) -> p r m", r=4),
                    in_=masks[:].rearrange("r p m -> p r m"),
                )

                qk_sb = bigpool.tile([128, S], F16, tag="qk")
                v_sb = bigpool.tile([128, NBLK * 65], F16, tag="v")
                qd = bigpool.tile([128, S // 2], F16, tag="qd")

                # ones columns for the fused softmax-denominator
                nc.vector.memset(v_sb[:], 1.0)

                # ---- projections ----
                for c in range(8):
                    xts = []
                    for i in range(4):
                        t = xtpool.tile([128, 512], F16, tag=f"xt{i}")
                        nc.sync.dma_start(
                            out=t[:],
                            in_=xt[i * 128 : (i + 1) * 128, c * 512 : (c + 1) * 512],
                        )
                        xts.append(t)
                    qkp = scpool.tile([128, 512], F32, tag="sc")
                    for i in range(4):
                        nc.tensor.matmul(
                            qkp[:],
                            lhsT=w_kq[:, i * 128 : (i + 1) * 128],
                            rhs=xts[i][:],
                            start=(i == 0),
                            stop=(i == 3),
                        )
                    nc.vector.tensor_copy(
                        qk_sb[:, c * 512 : (c + 1) * 512], qkp[:]
                    )
                    vp = scpool.tile([128, 256], F32, tag="sc")
                    for i4 in range(4):
                        for i in range(4):
                            nc.tensor.matmul(
                                vp[:, i4 * 64 : (i4 + 1) * 64],
                                lhsT=xts[i][:, i4 * 128 : (i4 + 1) * 128],
                                rhs=w_v[:, i * 64 : (i + 1) * 64],
                                start=(i == 0),
                                stop=(i == 3),
                            )
                    nc.vector.tensor_copy(
                        v_sb[:, c * 260 : (c + 1) * 260].rearrange(
                            "p (b m) -> p b m", m=65
                        )[:, :, 0:64],
                        vp[:].rearrange("p (b m) -> p b m", m=64),
                    )

                # own-query q^T copy to partitions 0:64
                nc.sync.dma_start(out=qd[0:64, :], in_=qk_sb[64:128, 0 : S // 2])

                # ---- attention sweeps ----
                for sw in (0, 1):
                    g0 = 4 * sw
                    psA = accpool.tile([128, 260], F32, tag="acc")
                    psB = accpool.tile([128, 260], F32, tag="acc")
                    rmax = 8 if sw == 0 else 16
                    kbs = [r for r in range(rmax)] + [16 + r for r in range(rmax)]
                    for kb in kbs:
                        r = kb % 16
                        g_lo = max(g0, r // 2)
                        n_g = g0 + 4 - g_lo
                        sc = scpool.tile([128, n_g * 256], F32, tag="sc")
                        for g in range(g_lo, g0 + 4):
                            nc.tensor.matmul(
                                sc[:, (g - g_lo) * 256 : (g - g_lo + 1) * 256],
                                lhsT=qk_sb[0:64, kb * 128 : (kb + 1) * 128],
                                rhs=qd[0:64, g * 256 : (g + 1) * 256],
                                start=True,
                                stop=True,
                            )
                        pt = ppool.tile([128, n_g * 256], F16, tag="p")
                        nc.scalar.activation(
                            pt[:], sc[:], mybir.ActivationFunctionType.Exp,
                            scale=0.125,
                        )
                        if r // 2 == g_lo and (sw == 0 or r >= 8):
                            slot = (0 if kb < 16 else 2) + (r % 2)
                            nc.vector.tensor_mul(
                                pt[:, 0:256],
                                pt[:, 0:256],
                                mk[:, slot * 256 : (slot + 1) * 256],
                            )
                        for g in range(g_lo, g0 + 4):
                            s = g - g0
                            pc = (g - g_lo) * 256
                            last = kb == 17 + 2 * g
                            for half, ps in ((0, psA), (1, psB)):
                                # One start=True per PSUM tile (bank): the HW
                                # has_written clear is bank-granular, so only
                                # the very first matmul into the tile may set
                                # it; later slots first-touch pending bytes
                                # and overwrite-then-accumulate correctly.
                                nc.tensor.matmul(
                                    ps[:, s * 65 : (s + 1) * 65],
                                    lhsT=pt[:, pc + half * 128 : pc + (half + 1) * 128],
                                    rhs=v_sb[:, kb * 65 : (kb + 1) * 65],
                                    start=(kb == 0 and g == g_lo),
                                    stop=last,
                                    skip_group_check=True,
                                )
                            if last:
                                for half, ps in ((0, psA), (1, psB)):
                                    rec = opool.tile([128, 1], F32, tag="rec")
                                    nc.vector.reciprocal(
                                        rec[:], ps[:, s * 65 + 64 : s * 65 + 65]
                                    )
                                    ob = opool.tile([128, 64], F32, tag="ob")
                                    nc.vector.tensor_scalar_mul(
                                        ob[:], ps[:, s * 65 : s * 65 + 64], rec[:]
                                    )
                                    m = 2 * g + half
                                    nc.sync.dma_start(
                                        out=out[m * 128 : (m + 1) * 128, :],
                                        in_=ob[:],
                                    )

            if reps == 1:
                body()
            else:
                with tc.For_i(0, reps, 1) as _i:
                    body(_i)

    if patch:
        split_waits(nc)
    return nc


def make_core_inputs(x, Wq, Wk, Wv):
    """Full inputs -> list of 8 per-core input dicts (+ scatter info)."""
    f16 = np.float16
    wkq = np.concatenate([Wk, Wq], axis=1).astype(f16)  # [512, 128], k first
    wv = Wv.astype(f16)
    triu = np.triu(np.ones((128, 128), np.float16))
    masks_h = {}
    for h in (0, 1):
        m = np.zeros((4, 128, 256), f16)
        m[0, :, 0:128] = triu
        m[0, :, 128:256] = 1.0
        m[1, :, 128:256] = triu
        if h == 0:
            m[2, :, 128:256] = 1.0
        else:
            m[2] = 1.0
            m[3, :, 128:256] = 1.0
        masks_h[h] = m
    in_maps = []
    for c in range(NCORES):
        b, h = c // 2, c % 2
        own = [2 * m + h for m in range(16)]
        other = [2 * m + (1 - h) for m in range(16)]
        cols = np.concatenate(
            [np.arange(g * 128, (g + 1) * 128) for g in own + other]
        )
        xtl = np.ascontiguousarray(x[b][cols].T.astype(f16))  # [512, 4096]
        in_maps.append(
            {"xt": xtl, "wkq": wkq, "wv": wv, "masks": masks_h[h]}
        )
    return in_maps


def scatter_outputs(results):
    """Per-core [2048, 64] outputs -> full [B, S, 64]."""
    out = np.zeros((B, S, DOUT), np.float32)
    for c in range(NCORES):
        b, h = c // 2, c % 2
        oc = results[c]["out"]
        for m in range(16):
            out[b, (2 * m + h) * 128 : (2 * m + h + 1) * 128] = oc[
                m * 128 : (m + 1) * 128
            ]
    return out


_cached = {}


def _get_program(reps=1):
    if reps not in _cached:
        _cached[reps] = build_program(reps)
    return _cached[reps]


def kernel(x, Wq, Wk, Wv):
    from concourse.bass_utils import run_bass_kernel_spmd

    x = np.asarray(x, np.float32)
    Wq = np.asarray(Wq, np.float32)
    Wk = np.asarray(Wk, np.float32)
    Wv = np.asarray(Wv, np.float32)
    nc = _get_program(1)
    in_maps = make_core_inputs(x, Wq, Wk, Wv)
    res = run_bass_kernel_spmd(nc, in_maps, core_ids=list(range(NCORES)))
    return scatter_outputs(res.results)


# revision 18
# speedup vs baseline: 15.8179x; 15.8179x over previous
"""Trainium2 Bass kernel for single-head causal attention.

Problem: B=4, S=4096, d_in=512, d_out=64 (fp32 reference).

Sharding (8 cores): core c = (batch b = c//2, query-parity h = c%2).
Each core handles one batch and the 16 query blocks of 128 with block
index === h (mod 2).  The host permutes the sequence dimension so each
core's x^T arrives as [own 2048 query columns | other 2048], which makes
the SPMD program identical across cores: all parity differences are
encoded in a per-core multiplicative mask input.

Device program per core:
  - project [Wk|Wq] against x^T chunks -> qk_sb [128, 4096] fp16
    (rows 0:64 = k^T, rows 64:128 = q^T, columns in local order)
  - project Wv -> v_sb [128, 32*65] fp16 (V blocks of [128, 64] plus a
    fused ones column per block for the softmax denominator)
  - attention in two sweeps of 4 query groups (group = 256 queries):
    for each key block kb: scores^T = k_kb^T.T @ q_group (PSUM, fp32),
    one Exp activation over all valid groups (scale = 1/8, no max
    subtraction -- scores are bounded), multiplicative causal masks on
    the <=1 masked group, then P-stationary matmuls accumulate
    out[q,0:64] = P.T @ V and out[q,64] = sum_k P into per-group PSUM
    slots.  Finalize = reciprocal + per-partition scale + DMA out.
"""

import os
import sys

sys.path.insert(0, "/opt/trn_rl_repo")

import numpy as np

import concourse.bass as bass
import concourse.mybir as mybir
import concourse.tile as tile

B, S, DIN, DOUT = 4, 4096, 512, 64
NCORES = 8
NBLK = S // 128          # 32 key blocks of 128
NGRP = 8                 # query groups of 256 (own queries only)
F16 = mybir.dt.float16
F32 = mybir.dt.float32


def split_waits(nc, maxw=1):
    """Walrus in this toolchain rejects >1 semaphore wait on ctrl-class
    instructions; hoist excess waits onto preceding same-engine NoOps."""
    ctr = 0
    for f in nc.m.functions:
        for bb in f.blocks:
            out = []
            for inst in bb.instructions:
                si = inst.sync_info
                waits = list(si.on_wait) if si and si.on_wait else []
                if len(waits) > maxw:
                    hoisted, rest = waits[:-maxw], waits[-maxw:]
                    for i in range(0, len(hoisted), maxw):
                        nop = mybir.InstNoOp(
                            name=f"waitsplit-{ctr}",
                            ins=[],
                            outs=[],
                            engine=inst.engine,
                            sync_info=mybir.SyncInfo(
                                on_wait=hoisted[i : i + maxw], on_update=[]
                            ),
                        )
                        ctr += 1
                        out.append(nop)
                    si.on_wait = rest
                out.append(inst)
            bb.instructions = out
    return ctr


def build_program(reps=1, patch=True):
    nc = bass.Bass("TRN2", target_bir_lowering=False, debug=False)

    xt = nc.dram_tensor("xt", [DIN, S], F16, kind="ExternalInput")
    wkq = nc.dram_tensor("wkq", [DIN, 128], F16, kind="ExternalInput")
    wv = nc.dram_tensor("wv", [DIN, DOUT], F16, kind="ExternalInput")
    masks = nc.dram_tensor("masks", [4, 128, 256], F16, kind="ExternalInput")
    out = nc.dram_tensor("out", [S // 2, DOUT], F32, kind="ExternalOutput")

    with tile.TileContext(nc) as tc:
        with (
            tc.tile_pool(name="const", bufs=1) as cpool,
            tc.tile_pool(name="big", bufs=1) as bigpool,
            tc.tile_pool(name="xt", bufs=4) as xtpool,
            tc.tile_pool(name="p", bufs=3) as ppool,
            tc.tile_pool(name="osb", bufs=3) as opool,
            tc.tile_pool(name="sc", bufs=3, space="PSUM") as scpool,
            tc.tile_pool(name="acc", bufs=2, space="PSUM") as accpool,
        ):
            def body(_iv=None):
                w_kq = cpool.tile([128, 512], F16, tag="wkq")
                nc.sync.dma_start(
                    out=w_kq[:].rearrange("p (i m) -> p i m", i=4),
                    in_=wkq[:].rearrange("(i p) m -> p i m", p=128),
                )
                w_v = cpool.tile([128, 256], F16, tag="wv")
                nc.sync.dma_start(
                    out=w_v[:].rearrange("p (i m) -> p i m", i=4),
                    in_=wv[:].rearrange("(i p) m -> p i m", p=128),
                )
                mk = cpool.tile([128, 1024], F16, tag="mk")

                # per-chunk tiles so attention can start before all
                # projections finish (Tile deps are tile-granular)
                qk_cs = [bigpool.tile([128, 512], F16, tag=f"qk{c}", name=f"qk{c}") for c in range(8)]
                v_cs = [bigpool.tile([128, 260], F16, tag=f"v{c}", name=f"v{c}") for c in range(8)]
                qd_cs = [bigpool.tile([128, 512], F16, tag=f"qd{c}", name=f"qd{c}") for c in range(4)]

                def kT(kb):  # [64, 128] slice for key block kb
                    return qk_cs[kb // 4][0:64, (kb % 4) * 128 : (kb % 4 + 1) * 128]

                def qT(g):  # [64, 256] slice for query group g
                    return qd_cs[g // 2][0:64, (g % 2) * 256 : (g % 2 + 1) * 256]

                def vaug(kb):  # [128, 65] slice for key block kb
                    return v_cs[kb // 4][:, (kb % 4) * 65 : (kb % 4 + 1) * 65]

                # ones columns for the fused softmax-denominator
                for c in range(8):
                    nc.vector.memset(v_cs[c][:], 1.0)

                # ---- projections (emitted per-chunk; attention for
                # sweep 0 is interleaved between late chunks so PE/ACT
                # overlap the input DMAs) ----
                def dma_chunk(c):
                    xc = xtpool.tile([128, 2048], F16, tag="xt", name=f"xc{c}")
                    nc.sync.dma_start(
                        out=xc[:].rearrange("p (i m) -> p i m", i=4),
                        in_=xt[:, c * 512 : (c + 1) * 512].rearrange(
                            "(i p) m -> p i m", p=128
                        ),
                    )
                    return xc

                def proj_chunk(c, xc):
                    xts = [xc[:, i * 512 : (i + 1) * 512] for i in range(4)]
                    qkp = scpool.tile([128, 512], F32, tag="sc", name=f"qkp{c}")
                    for i in range(4):
                        nc.tensor.matmul(
                            qkp[:],
                            lhsT=w_kq[:, i * 128 : (i + 1) * 128],
                            rhs=xts[i],
                            start=(i == 0),
                            stop=(i == 3),
                        )
                    nc.vector.tensor_copy(qk_cs[c][:], qkp[:])
                    vp = scpool.tile([128, 256], F32, tag="sc", name=f"vp{c}")
                    for i4 in range(4):
                        for i in range(4):
                            nc.tensor.matmul(
                                vp[:, i4 * 64 : (i4 + 1) * 64],
                                lhsT=xts[i][:, i4 * 128 : (i4 + 1) * 128],
                                rhs=w_v[:, i * 64 : (i + 1) * 64],
                                start=(i == 0),
                                stop=(i == 3),
                            )
                    nc.vector.tensor_copy(
                        v_cs[c][:].rearrange("p (b m) -> p b m", m=65)[:, :, 0:64],
                        vp[:].rearrange("p (b m) -> p b m", m=64),
                    )
                    if c < 4:
                        # own-query q^T for this chunk to partitions 0:64
                        nc.sync.dma_start(
                            out=qd_cs[c][0:64, :], in_=qk_cs[c][64:128, :]
                        )

                # stage the first four chunk DMAs, project what sweep 0
                # kbs 0-3 need (chunks 0,1 for kT/qT), then interleave the
                # remaining projections between attention bursts, always
                # emitting a projection before the kbs that consume it.
                PROJ_STAGE = (0, 1, 4, 5)
                xcs = {c: dma_chunk(c) for c in PROJ_STAGE}
                proj_chunk(0, xcs.pop(0))
                nc.sync.dma_start(
                    out=mk[:].rearrange("p (r m) -> p r m", r=4),
                    in_=masks[:].rearrange("r p m -> p r m"),
                )
                proj_chunk(1, xcs.pop(1))

                sweep_state = {}

                def sweep_open(sw):
                    g0 = 4 * sw
                    psA = accpool.tile([128, 260], F32, tag="acc", name=f"psA{sw}")
                    psB = accpool.tile([128, 260], F32, tag="acc", name=f"psB{sw}")
                    sweep_state[sw] = (g0, psA, psB)

                def emit_kbs(sw, kbs):
                    g0, psA, psB = sweep_state[sw]
                    for kb in kbs:
                        r = kb % 16
                        g_lo = max(g0, r // 2)
                        n_g = g0 + 4 - g_lo
                        sc = scpool.tile([128, n_g * 256], F32, tag="sc")
                        for g in range(g_lo, g0 + 4):
                            nc.tensor.matmul(
                                sc[:, (g - g_lo) * 256 : (g - g_lo + 1) * 256],
                                lhsT=kT(kb),
                                rhs=qT(g),
                                start=True,
                                stop=True,
                            )
                        pt = ppool.tile([128, n_g * 256], F16, tag="p")
                        nc.scalar.activation(
                            pt[:], sc[:], mybir.ActivationFunctionType.Exp,
                            scale=0.125,
                        )
                        if r // 2 == g_lo and (sw == 0 or r >= 8):
                            slot = (0 if kb < 16 else 2) + (r % 2)
                            nc.vector.tensor_mul(
                                pt[:, 0:256],
                                pt[:, 0:256],
                                mk[:, slot * 256 : (slot + 1) * 256],
                            )
                        for g in range(g_lo, g0 + 4):
                            s = g - g0
                            pc = (g - g_lo) * 256
                            last = kb == 17 + 2 * g
                            for half, ps in ((0, psA), (1, psB)):
                                # One start=True per PSUM tile (bank): the HW
                                # has_written clear is bank-granular.
                                nc.tensor.matmul(
                                    ps[:, s * 65 : (s + 1) * 65],
                                    lhsT=pt[:, pc + half * 128 : pc + (half + 1) * 128],
                                    rhs=vaug(kb),
                                    start=(kb == 0 and g == g_lo),
                                    stop=last,
                                    skip_group_check=True,
                                )
                            if last:
                                ob = opool.tile([128, 128], F32, tag="ob")
                                for half, ps in ((0, psA), (1, psB)):
                                    rec = opool.tile([128, 1], F32, tag="rec")
                                    nc.vector.reciprocal(
                                        rec[:], ps[:, s * 65 + 64 : s * 65 + 65]
                                    )
                                    nc.vector.tensor_scalar_mul(
                                        ob[:, half * 64 : (half + 1) * 64],
                                        ps[:, s * 65 : s * 65 + 64],
                                        rec[:],
                                    )
                                nc.sync.dma_start(
                                    out=out[2 * g * 128 : (2 * g + 2) * 128, :].rearrange(
                                        "(two p) m -> p two m", p=128
                                    ),
                                    in_=ob[:].rearrange("p (two m) -> p two m", two=2),
                                )

                def proj(c):
                    if c not in xcs:
                        xcs[c] = dma_chunk(c)
                    proj_chunk(c, xcs.pop(c))

                sweep_open(0)
                emit_kbs(0, [0, 1, 2, 3])
                proj(4)
                emit_kbs(0, [16, 17, 18])
                proj(5)
                emit_kbs(0, [19, 4, 5])
                proj(2)
                emit_kbs(0, [6, 7, 20])
                proj(3)
                emit_kbs(0, [21, 22, 23])
                proj(6)
                sweep_open(1)
                emit_kbs(1, [0, 1, 2, 3])
                proj(7)
                emit_kbs(1, [4, 5, 6, 7, 8, 9, 10, 11, 12, 13, 14, 15])
                emit_kbs(1, [16 + r for r in range(16)])

            if reps == 1:
                body()
            else:
                with tc.For_i(0, reps, 1) as _i:
                    body(_i)

    if patch:
        split_waits(nc)
    return nc


def make_core_inputs(x, Wq, Wk, Wv):
    """Full inputs -> list of 8 per-core input dicts (+ scatter info)."""
    f16 = np.float16
    wkq = np.concatenate([Wk, Wq], axis=1).astype(f16)  # [512, 128], k first
    wv = Wv.astype(f16)
    triu = np.triu(np.ones((128, 128), np.float16))
    masks_h = {}
    for h in (0, 1):
        m = np.zeros((4, 128, 256), f16)
        m[0, :, 0:128] = triu
        m[0, :, 128:256] = 1.0
        m[1, :, 128:256] = triu
        if h == 0:
            m[2, :, 128:256] = 1.0
        else:
            m[2] = 1.0
            m[3, :, 128:256] = 1.0
        masks_h[h] = m
    in_maps = []
    for c in range(NCORES):
        b, h = c // 2, c % 2
        own = [2 * m + h for m in range(16)]
        other = [2 * m + (1 - h) for m in range(16)]
        cols = np.concatenate(
            [np.arange(g * 128, (g + 1) * 128) for g in own + other]
        )
        xtl = np.ascontiguousarray(x[b][cols].T.astype(f16))  # [512, 4096]
        in_maps.append(
            {"xt": xtl, "wkq": wkq, "wv": wv, "masks": masks_h[h]}
        )
    return in_maps


def scatter_outputs(results):
    """Per-core [2048, 64] outputs -> full [B, S, 64]."""
    out = np.zeros((B, S, DOUT), np.float32)
    for c in range(NCORES):
        b, h = c // 2, c % 2
        oc = results[c]["out"]
        for m in range(16):
            out[b, (2 * m + h) * 128 : (2 * m + h + 1) * 128] = oc[
                m * 128 : (m + 1) * 128
            ]
    return out


_cached = {}


def _get_program(reps=1):
    if reps not in _cached:
        _cached[reps] = build_program(reps)
    return _cached[reps]


def kernel(x, Wq, Wk, Wv):
    from concourse.bass_utils import run_bass_kernel_spmd

    x = np.asarray(x, np.float32)
    Wq = np.asarray(Wq, np.float32)
    Wk = np.asarray(Wk, np.float32)
    Wv = np.asarray(Wv, np.float32)
    nc = _get_program(1)
    in_maps = make_core_inputs(x, Wq, Wk, Wv)
    res = run_bass_kernel_spmd(nc, in_maps, core_ids=list(range(NCORES)))
    return scatter_outputs(res.results)
